# revision 21
# baseline (speedup 1.0000x reference)
"""Causal attention (B=2, H=16, S=2048, D=64, f32) on 8 TRN2 NeuronCores.

Sharding: the 32 (batch, head) pairs are split 4-per-core (pure data/head
parallelism, no collectives). Host passes Q and K pre-transposed to [d, q]
layout and pre-cast to bf16; V likewise bf16 in natural [k, d] layout.

Per (b,h) pair, key chunks are processed two at a time (j even/odd).
The scores PSUM buffer is ONE [128, 2048] f32 tile used as FOUR
single-bank slots holding 256-query windows: window u puts the even
chunk in bank 2*pb (half u%2) and the odd chunk in bank 2*pb+1, so the
two tile_position-concurrent QK matmuls never share a PSUM bank and the
QK stream runs a full bank-pair (2 windows + an exp) ahead of the exp
drain -- the QK -> exp -> QK slot round-trip stays off the critical path.

  P^T = exp(scale * scores^T)    one instruction per bank pair (1024
      cols), assigned WHOLE to one engine: ~5/8 of units to the exact
      ACT exp, ~3/8 to the Vector engine via a one-instruction
      Schraudolph: i16 = rint(x*SCALE*128/ln2 + (127*128 - 7.4)), whose
      bits ARE bf16 exp(x*SCALE) to ~3% rms (the -7.4 cancels the
      systematic (1+f) vs 2^f overshoot so mixing engines across a query
      row stays unbiased; softmax renormalization absorbs the rest ->
      ~1e-2 output rel err, under the 2e-2 gate). Whole-unit assignment
      keeps each slot release on a SINGLE engine semaphore, so ACT and
      DVE drain different bank pairs truly concurrently and the former
      single-engine softmax bottleneck (~75us of ACT) splits across both.
  P^T[:, diag] *= tri01          post-exp causal mask, both chunks in ONE
      strided DVE op, emitted at PV-flush time (2 chunk-pairs after its
      inputs) so it never head-of-line-blocks the DVE queue.
  acc[q, :] += P_block^T.T @ [V_j | 1 | 0pad]   P^T blocks as weights so
      the output lands directly in [q, d] layout; column 64 accumulates
      the softmax denominator via the ones column; zero-pad keeps the PE
      duty-cycle high so the HAM clock-gate stays at 2.4 GHz.
  out[q, :] = acc[q, 0:64] * (1 / acc[q, 64])   computed straight from
      PSUM (no staging copy), two PSUM banks (8 q-tiles) per pass.

The PV matmuls trail the QK/exp stream by two chunk-pairs and are spread
between QK window emissions, forming one global software pipeline across
all 4 (b,h) pairs.
"""

import os
import sys

# Precise (non-budgeted) region-overlap analysis in the Tile dependency
# tracker: the rotating single-tile PSUM slot scheme relies on fine-grained
# AP intersection; the default work-budget falls back to assume-overlap and
# serializes the pipeline (~2x slower).
os.environ.setdefault("TILE_EXHAUSTIVE_MEMORY_SHARE_CHECK", "1")

if "/opt/trn_rl_repo" not in sys.path:
    sys.path.insert(0, "/opt/trn_rl_repo")

from contextlib import ExitStack

import ml_dtypes
import numpy as np

import concourse.bass as bass
import concourse.bacc as bacc
import concourse.tile as tile
from concourse import mybir
from concourse.bass_utils import run_bass_kernel_spmd

B, H, S, D = 2, 16, 2048, 64
NCORES = 8
PAIRS = (B * H) // NCORES  # 4 (b,h) pairs per core
NT = S // 128  # 16 key chunks / query tiles
F32 = mybir.dt.float32
BF16 = mybir.dt.bfloat16
I16 = mybir.dt.int16
SCALE = 0.125  # 1/sqrt(D)
PV_N = int(os.environ.get("PV_N", "128"))  # PV stream width (65..128)
WARMUP = int(os.environ.get("WARMUP", "8"))
# which exp units (mod 8) take the DVE fast-exp path
DVE_SEGS = frozenset(
    int(x) for x in os.environ.get("DVE_SEGS", "1,4,6").split(",") if x != ""
)
# Schraudolph constants: i16 = rint(score * C1S + C2S) bitcast to bf16
C1S = float(SCALE * 128.0 / np.log(2.0))
C2S = float(127.0 * 128.0 - 7.4)


def build_nc():
    nc = bacc.Bacc(None)
    qT = nc.declare_dram_parameter("qT", [PAIRS, D, S], BF16, isOutput=False)
    kT = nc.declare_dram_parameter("kT", [PAIRS, D, S], BF16, isOutput=False)
    v = nc.declare_dram_parameter("v", [PAIRS, S, D], BF16, isOutput=False)
    out = nc.declare_dram_parameter("out", [PAIRS, S, D], F32, isOutput=True)

    with tile.TileContext(nc) as tc, ExitStack() as ctx:
        consts = ctx.enter_context(tc.tile_pool(name="consts", bufs=1))
        qtp = ctx.enter_context(tc.tile_pool(name="qt", bufs=2))
        ktp = ctx.enter_context(tc.tile_pool(name="kt", bufs=2))
        vpp = ctx.enter_context(tc.tile_pool(name="vp", bufs=2))
        ptp = ctx.enter_context(tc.tile_pool(name="pt", bufs=3))
        outp = ctx.enter_context(tc.tile_pool(name="outsb", bufs=2))
        smalls = ctx.enter_context(tc.tile_pool(name="smalls", bufs=4))
        ps_scores = ctx.enter_context(
            tc.tile_pool(name="ps_scores", bufs=1, space="PSUM")
        )
        ps_acc = ctx.enter_context(tc.tile_pool(name="ps_acc", bufs=1, space="PSUM"))

        # tri01[k_local, q_local] = 1 where q >= k else 0 (bf16, post-exp mask)
        tri01 = consts.tile([128, 128], BF16)
        nc.gpsimd.memset(tri01, 1.0)
        nc.gpsimd.affine_select(
            out=tri01,
            in_=tri01,
            compare_op=mybir.AluOpType.is_ge,
            fill=0.0,
            base=0,
            pattern=[[1, 128]],
            channel_multiplier=-1,
        )

        # the whole-kernel scores buffer: 4 banks = 2 rotating bank pairs
        sc = ps_scores.tile([128, 2048], F32, name="sc")

        # PE warm-up during the first DMAs: the HAM clock-gate starts at
        # 1.2 GHz and needs ~3.4us of continuous PE activity to release.
        t01 = tri01[:, :]
        tri_rep = bass.AP(
            tensor=t01.tensor,
            offset=t01.offset,
            ap=[t01.ap[0], [0, 4], t01.ap[1]],
        )
        for _ in range(WARMUP):
            nc.tensor.matmul(sc[:, 0:512], tri01, tri_rep, start=True, stop=True)
        # preload the ACT exp table set (~2.7us) while the first DMAs run
        tbl = smalls.tile([128, 1], F32, tag="rec", name="tbl")
        nc.scalar.activation(tbl, tri01[:, 0:1], mybir.ActivationFunctionType.Exp)

        def load_pair(p):
            # Q^T/K^T duplicated onto partitions 64-127 so even/odd key
            # chunks can use disjoint halves of the PE array.
            qt = qtp.tile([128, S], BF16, tag="qt")
            kt = ktp.tile([128, S], BF16, tag="kt")
            vp_t = vpp.tile([128, NT, 128], BF16, tag="vp")
            # three DMA stages so the first QK windows (q < 512, chunks 0/1)
            # wait only on a small head transfer
            hq0, hq, hk = 512, 1024, 256
            for r0 in (0, D):
                nc.sync.dma_start(out=qt[r0 : r0 + D, 0:hq0], in_=qT[p][:, 0:hq0])
                nc.sync.dma_start(out=kt[r0 : r0 + D, 0:hk], in_=kT[p][:, 0:hk])
            for r0 in (0, D):
                nc.sync.dma_start(out=qt[r0 : r0 + D, hq0:hq], in_=qT[p][:, hq0:hq])
                nc.sync.dma_start(out=kt[r0 : r0 + D, hk:], in_=kT[p][:, hk:])
            for r0 in (0, D):
                nc.sync.dma_start(out=qt[r0 : r0 + D, hq:], in_=qT[p][:, hq:])
            nc.sync.dma_start(
                out=vp_t[:, :, 0:D],
                in_=v[p].rearrange("(t pp) d -> pp t d", pp=128),
            )
            if p < 2:
                # ones column (denominator) and zero pad: the vp pool has 2
                # rotating buffers, so pairs 2/3 inherit these from pairs 0/1
                # (their DMAs only overwrite cols 0:D).
                nc.vector.memset(vp_t[:, :, D : D + 1], 1.0)
                nc.vector.memset(vp_t[:, :, D + 1 :], 0.0)
            return {
                "qt": qt,
                "kt": kt,
                "vp": vp_t,
                "acc": None,
                "out_r": out[p].rearrange("(t pp) d -> pp t d", pp=128),
            }

        def pcol(j, c):
            # ptab column of chunk j's local-q position c: per 1024-col exp
            # unit t, layout [even(2t) | even(2t+1) | odd(2t) | odd(2t+1)]
            u = c // 256
            return 1024 * (u // 2) + 512 * (j % 2) + 256 * (u % 2) + (c % 256)

        def emit_qk(st, ja, jb, u, pb):
            # one 256-wide window of each chunk; the even chunk lands in
            # bank 2*pb (half u%2), the odd chunk in bank 2*pb+1, so the two
            # tile_position-concurrent matmuls never share a PSUM bank.
            qt, kt = st["qt"], st["kt"]
            for r0, j in ((0, ja), (D, jb)):
                q0 = j * 128 + 256 * u
                w = min(256, S - q0)
                base = 1024 * pb + 512 * (1 if r0 else 0) + 256 * (u % 2)
                nc.tensor.matmul(
                    sc[:, base : base + w],
                    kt[r0 : r0 + D, j * 128 : j * 128 + 128],
                    qt[r0 : r0 + D, q0 : q0 + w],
                    start=True,
                    stop=True,
                    tile_position=(r0, 0),
                )

        def emit_exp(ptab, t, pb, full, dve):
            # exp of one bank pair (two 256-windows of both chunks) on a
            # SINGLE engine, so the slot release rides one semaphore. When
            # full=False only the first window of each bank is live (odd
            # nseg tail): a 3D AP skips the second halves.
            if full:

                def rng(a):
                    return a  # flat [128, 1024]

                pi = sc[:, 1024 * pb : 1024 * pb + 1024]
                po = ptab[:, 1024 * t : 1024 * t + 1024]
            else:
                pi0 = sc[:, 1024 * pb : 1024 * pb + 256]
                po0 = ptab[:, 1024 * t : 1024 * t + 256]

                def mk(a):
                    return bass.AP(
                        tensor=a.tensor,
                        offset=a.offset,
                        ap=[a.ap[0], [512, 2], [1, 256]],
                    )

                pi = mk(pi0)
                po = mk(po0)
            if dve:
                nc.vector.tensor_scalar(
                    out=po.bitcast(I16),
                    in0=pi,
                    scalar1=C1S,
                    scalar2=C2S,
                    op0=mybir.AluOpType.mult,
                    op1=mybir.AluOpType.add,
                )
            else:
                nc.scalar.activation(
                    po, pi, mybir.ActivationFunctionType.Exp, scale=SCALE
                )

        def emit_trimask(ptab):
            # causal mask on both chunks' diagonal blocks in one op; emitted
            # at flush time (2 chunk-pairs after the exps it reads) so it
            # executes immediately and never head-of-line-blocks the DVE.
            pt0 = ptab[:, 0:128]
            ap2 = bass.AP(
                tensor=pt0.tensor,
                offset=pt0.offset,
                ap=[pt0.ap[0], [512, 2], [1, 128]],
            )
            nc.vector.tensor_mul(
                ap2,
                ap2,
                bass.AP(
                    tensor=t01.tensor,
                    offset=t01.offset,
                    ap=[t01.ap[0], [0, 2], t01.ap[1]],
                ),
            )

        def pv_mms(st, j, ptab):
            acc = st["acc"]
            for i in range(j, NT):
                c = pcol(j, (i - j) * 128)
                # start=True clears the whole PSUM *bank* (4 acc regions), so
                # only the first region touched per bank may set it.
                yield (
                    acc[:, i, 0:PV_N],
                    ptab[:, c : c + 128],
                    st["vp"][:, j, 0:PV_N],
                    j == 0 and i % 4 == 0,
                    j == i,
                )

        def emit_pv_mm(mm):
            o, l, r, st_, sp = mm
            nc.tensor.matmul(o, l, r, start=st_, stop=sp)

        def emit_finish(st, g):
            # normalize/store 8 finished q-tiles (two PSUM banks) straight
            # from PSUM: q-tile i gets its last PV contribution at chunk
            # j=i, so banks 2g,2g+1 are final once chunk 8g+7's PV is done.
            acc = st["acc"]
            g0 = 8 * g
            rec8 = smalls.tile([128, 8], F32, tag="rec")
            nc.vector.reciprocal(rec8, acc[:, g0 : g0 + 8, D])
            osb = outp.tile([128, 8, D], F32, tag="osb")
            r8 = rec8[:, :]
            rec_bcast = bass.AP(
                tensor=r8.tensor,
                offset=r8.offset,
                ap=[r8.ap[0], r8.ap[1], [0, D]],
            )
            nc.vector.tensor_mul(osb, acc[:, g0 : g0 + 8, 0:D], rec_bcast)
            nc.sync.dma_start(out=st["out_r"][:, g0 : g0 + 8, :], in_=osb)

        # ---- one global pipeline over all (pair, chunk-pair) units ----
        states = [None] * PAIRS
        states[0] = load_pair(0)
        pending = []  # (state, ja, jb, ptab) whose PV is not yet emitted
        gseg = 0  # global window counter (slot rotation)
        eunit = 0  # global exp-unit counter (engine assignment)

        def flush_one():
            fst, oa, ob, opab = pending.pop(0)
            emit_trimask(opab)
            pv = list(pv_mms(fst, oa, opab)) + list(pv_mms(fst, ob, opab))
            fin = ob // 8 if ob % 8 == 7 else None
            return fst, pv, fin

        for p in range(PAIRS):
            st = states[p]
            if p + 1 < PAIRS:
                states[p + 1] = load_pair(p + 1)
            st["acc"] = ps_acc.tile([128, NT, 128], F32, tag="acc", name="acc_t")
            for jp in range(0, NT, 2):
                ja, jb = jp, jp + 1
                ptab = ptp.tile([128, 4096], BF16, tag="pt")
                nseg = (S - ja * 128 + 255) // 256
                if gseg % 2 == 1:
                    gseg += 1  # start each chunk-pair on an even window
                s0 = gseg % 4
                pv, fin, fst = [], None, None
                depth = 1 if (p == PAIRS - 1 and jp >= NT - 4) else 2
                if len(pending) >= depth:
                    fst, pv, fin = flush_one()
                nunits = (nseg + 1) // 2
                per_unit = (len(pv) + nunits - 1) // nunits if pv else 0
                k = 0
                for u in range(nseg):
                    # both QK windows of a unit go to the PE queue BEFORE the
                    # PV fill-in batch, so the exp's inputs are never stuck
                    # behind PV matmuls and the exp starts ~0.5us earlier.
                    emit_qk(st, ja, jb, u, ((s0 + u) // 2) % 2)
                    if u % 2 == 1:
                        emit_exp(
                            ptab,
                            u // 2,
                            ((s0 + u) // 2) % 2,
                            True,
                            eunit % 8 in DVE_SEGS,
                        )
                        eunit += 1
                        take = (
                            pv[k : k + per_unit] if u < nseg - 2 else pv[k:]
                        )
                        for mm in take:
                            emit_pv_mm(mm)
                        k += len(take)
                    gseg += 1
                if nseg % 2 == 1:
                    emit_exp(
                        ptab,
                        nseg // 2,
                        ((s0 + nseg - 1) // 2) % 2,
                        False,
                        eunit % 8 in DVE_SEGS,
                    )
                    eunit += 1
                    for mm in pv[k:]:
                        emit_pv_mm(mm)
                    k = len(pv)
                if fin is not None:
                    emit_finish(fst, fin)
                pending.append((st, ja, jb, ptab))
        while pending:
            fst, pv, fin = flush_one()
            for mm in pv:
                emit_pv_mm(mm)
            if fin is not None:
                emit_finish(fst, fin)
    nc.compile()
    return nc


_nc_cache = None


def _get_nc():
    global _nc_cache
    if _nc_cache is None:
        _nc_cache = build_nc()
    return _nc_cache


def kernel(q, k, v, mask):
    """Full causal attention. q,k,v: [B,H,S,D] f32; mask: [1,1,S,S] bool
    (causal tril; baked into the kernel). Returns [B,H,S,D] f32."""
    nc = _get_nc()
    bf = ml_dtypes.bfloat16
    qf = np.asarray(q, dtype=np.float32).reshape(B * H, S, D)
    kf = np.asarray(k, dtype=np.float32).reshape(B * H, S, D)
    vf = np.ascontiguousarray(
        np.asarray(v, dtype=np.float32).reshape(B * H, S, D).astype(bf)
    )
    qTf = np.ascontiguousarray(qf.transpose(0, 2, 1).astype(bf))
    kTf = np.ascontiguousarray(kf.transpose(0, 2, 1).astype(bf))
    in_maps = [
        {
            "qT": qTf[i * PAIRS : (i + 1) * PAIRS],
            "kT": kTf[i * PAIRS : (i + 1) * PAIRS],
            "v": vf[i * PAIRS : (i + 1) * PAIRS],
        }
        for i in range(NCORES)
    ]
    res = run_bass_kernel_spmd(nc, in_maps, core_ids=list(range(NCORES)))
    o = np.concatenate([res.results[i]["out"] for i in range(NCORES)], axis=0)
    return o.reshape(B, H, S, D)


# revision 22
# speedup vs baseline: 1.0230x; 1.0230x over previous
"""Causal attention (B=2, H=16, S=2048, D=64, f32) on 8 TRN2 NeuronCores.

Sharding: the 32 (batch, head) pairs are split 4-per-core (pure data/head
parallelism, no collectives). Host passes Q and K pre-transposed to [d, q]
layout and pre-cast to bf16; V likewise bf16 in natural [k, d] layout.

Per (b,h) pair, key chunks are processed two at a time (j even/odd).
The scores PSUM buffer is ONE [128, 2048] f32 tile used as FOUR
single-bank slots holding 256-query windows: window u puts the even
chunk in bank 2*pb (half u%2) and the odd chunk in bank 2*pb+1, so the
two tile_position-concurrent QK matmuls never share a PSUM bank and the
QK stream runs a full bank-pair (2 windows + an exp) ahead of the exp
drain -- the QK -> exp -> QK slot round-trip stays off the critical path.

  P^T = exp(scale * scores^T)    one instruction per bank pair (1024
      cols), assigned WHOLE to one engine: ~5/8 of units to the exact
      ACT exp, ~3/8 to the Vector engine via a one-instruction
      Schraudolph: i16 = rint(x*SCALE*128/ln2 + (127*128 - 7.4)), whose
      bits ARE bf16 exp(x*SCALE) to ~3% rms (the -7.4 cancels the
      systematic (1+f) vs 2^f overshoot so mixing engines across a query
      row stays unbiased; softmax renormalization absorbs the rest ->
      ~1e-2 output rel err, under the 2e-2 gate). Whole-unit assignment
      keeps each slot release on a SINGLE engine semaphore, so ACT and
      DVE drain different bank pairs truly concurrently and the former
      single-engine softmax bottleneck (~75us of ACT) splits across both.
  P^T[:, diag] *= tri01          post-exp causal mask, both chunks in ONE
      strided DVE op, emitted at PV-flush time (2 chunk-pairs after its
      inputs) so it never head-of-line-blocks the DVE queue.
  acc[q, :] += P_block^T.T @ [V_j | 1 | 0pad]   P^T blocks as weights so
      the output lands directly in [q, d] layout; column 64 accumulates
      the softmax denominator via the ones column; zero-pad keeps the PE
      duty-cycle high so the HAM clock-gate stays at 2.4 GHz.
  out[q, :] = acc[q, 0:64] * (1 / acc[q, 64])   computed straight from
      PSUM (no staging copy), two PSUM banks (8 q-tiles) per pass.

The PV matmuls trail the QK/exp stream by two chunk-pairs and are spread
between QK window emissions, forming one global software pipeline across
all 4 (b,h) pairs.
"""

import os
import sys

# Precise (non-budgeted) region-overlap analysis in the Tile dependency
# tracker: the rotating single-tile PSUM slot scheme relies on fine-grained
# AP intersection; the default work-budget falls back to assume-overlap and
# serializes the pipeline (~2x slower).
os.environ.setdefault("TILE_EXHAUSTIVE_MEMORY_SHARE_CHECK", "1")

if "/opt/trn_rl_repo" not in sys.path:
    sys.path.insert(0, "/opt/trn_rl_repo")

from contextlib import ExitStack

import ml_dtypes
import numpy as np

import concourse.bass as bass
import concourse.bacc as bacc
import concourse.tile as tile
from concourse import mybir
from concourse.bass_utils import run_bass_kernel_spmd

B, H, S, D = 2, 16, 2048, 64
NCORES = 8
PAIRS = (B * H) // NCORES  # 4 (b,h) pairs per core
NT = S // 128  # 16 key chunks / query tiles
F32 = mybir.dt.float32
BF16 = mybir.dt.bfloat16
I16 = mybir.dt.int16
SCALE = 0.125  # 1/sqrt(D)
PV_N = int(os.environ.get("PV_N", "128"))  # PV stream width (65..128)
WARMUP = int(os.environ.get("WARMUP", "8"))
# which exp units (mod 8) take the DVE fast-exp path
DVE_SEGS = frozenset(
    int(x) for x in os.environ.get("DVE_SEGS", "1,4,6").split(",") if x != ""
)
# Schraudolph constants: i16 = rint(score * C1S + C2S) bitcast to bf16
C1S = float(SCALE * 128.0 / np.log(2.0))
C2S = float(127.0 * 128.0 - 7.4)


def build_nc():
    nc = bacc.Bacc(None)
    qT = nc.declare_dram_parameter("qT", [PAIRS, D, S], BF16, isOutput=False)
    kT = nc.declare_dram_parameter("kT", [PAIRS, D, S], BF16, isOutput=False)
    v = nc.declare_dram_parameter("v", [PAIRS, S, D], BF16, isOutput=False)
    out = nc.declare_dram_parameter("out", [PAIRS, S, D], F32, isOutput=True)

    with tile.TileContext(nc) as tc, ExitStack() as ctx:
        consts = ctx.enter_context(tc.tile_pool(name="consts", bufs=1))
        qtp = ctx.enter_context(tc.tile_pool(name="qt", bufs=2))
        ktp = ctx.enter_context(tc.tile_pool(name="kt", bufs=2))
        vpp = ctx.enter_context(tc.tile_pool(name="vp", bufs=2))
        ptp = ctx.enter_context(tc.tile_pool(name="pt", bufs=3))
        outp = ctx.enter_context(tc.tile_pool(name="outsb", bufs=2))
        smalls = ctx.enter_context(tc.tile_pool(name="smalls", bufs=4))
        ps_scores = ctx.enter_context(
            tc.tile_pool(name="ps_scores", bufs=1, space="PSUM")
        )
        ps_acc = ctx.enter_context(tc.tile_pool(name="ps_acc", bufs=1, space="PSUM"))

        # tri01[k_local, q_local] = 1 where q >= k else 0 (bf16, post-exp mask)
        tri01 = consts.tile([128, 128], BF16)
        nc.gpsimd.memset(tri01, 1.0)
        nc.gpsimd.affine_select(
            out=tri01,
            in_=tri01,
            compare_op=mybir.AluOpType.is_ge,
            fill=0.0,
            base=0,
            pattern=[[1, 128]],
            channel_multiplier=-1,
        )

        # the whole-kernel scores buffer: 4 banks = 2 rotating bank pairs
        sc = ps_scores.tile([128, 2048], F32, name="sc")

        # PE warm-up during the first DMAs: the HAM clock-gate starts at
        # 1.2 GHz and needs ~3.4us of continuous PE activity to release.
        t01 = tri01[:, :]
        tri_rep = bass.AP(
            tensor=t01.tensor,
            offset=t01.offset,
            ap=[t01.ap[0], [0, 4], t01.ap[1]],
        )
        for _ in range(WARMUP):
            nc.tensor.matmul(sc[:, 0:512], tri01, tri_rep, start=True, stop=True)
        # preload the ACT exp table set (~2.7us) while the first DMAs run
        tbl = smalls.tile([128, 1], F32, tag="rec", name="tbl")
        nc.scalar.activation(tbl, tri01[:, 0:1], mybir.ActivationFunctionType.Exp)

        def load_pair(p):
            # Q^T/K^T duplicated onto partitions 64-127 so even/odd key
            # chunks can use disjoint halves of the PE array.
            qt = qtp.tile([128, S], BF16, tag="qt")
            kt = ktp.tile([128, S], BF16, tag="kt")
            vp_t = vpp.tile([128, NT, 128], BF16, tag="vp")
            hq, hk = 1024, 256
            for r0 in (0, D):
                nc.sync.dma_start(out=qt[r0 : r0 + D, 0:hq], in_=qT[p][:, 0:hq])
                nc.sync.dma_start(out=kt[r0 : r0 + D, 0:hk], in_=kT[p][:, 0:hk])
            for r0 in (0, D):
                nc.sync.dma_start(out=qt[r0 : r0 + D, hq:], in_=qT[p][:, hq:])
                nc.sync.dma_start(out=kt[r0 : r0 + D, hk:], in_=kT[p][:, hk:])
            nc.sync.dma_start(
                out=vp_t[:, :, 0:D],
                in_=v[p].rearrange("(t pp) d -> pp t d", pp=128),
            )
            if p < 2:
                # ones column (denominator) and zero pad: the vp pool has 2
                # rotating buffers, so pairs 2/3 inherit these from pairs 0/1
                # (their DMAs only overwrite cols 0:D).
                nc.vector.memset(vp_t[:, :, D : D + 1], 1.0)
                nc.vector.memset(vp_t[:, :, D + 1 :], 0.0)
            return {
                "qt": qt,
                "kt": kt,
                "vp": vp_t,
                "acc": None,
                "out_r": out[p].rearrange("(t pp) d -> pp t d", pp=128),
            }

        def pcol(j, c):
            # ptab column of chunk j's local-q position c: per 1024-col exp
            # unit t, layout [even(2t) | even(2t+1) | odd(2t) | odd(2t+1)]
            u = c // 256
            return 1024 * (u // 2) + 512 * (j % 2) + 256 * (u % 2) + (c % 256)

        def emit_qk(st, ja, jb, u, pb):
            # one 256-wide window of each chunk; the even chunk lands in
            # bank 2*pb (half u%2), the odd chunk in bank 2*pb+1, so the two
            # tile_position-concurrent matmuls never share a PSUM bank.
            qt, kt = st["qt"], st["kt"]
            for r0, j in ((0, ja), (D, jb)):
                q0 = j * 128 + 256 * u
                w = min(256, S - q0)
                base = 1024 * pb + 512 * (1 if r0 else 0) + 256 * (u % 2)
                nc.tensor.matmul(
                    sc[:, base : base + w],
                    kt[r0 : r0 + D, j * 128 : j * 128 + 128],
                    qt[r0 : r0 + D, q0 : q0 + w],
                    start=True,
                    stop=True,
                    tile_position=(r0, 0),
                )

        def emit_exp(ptab, t, pb, full, dve):
            # exp of one bank pair (two 256-windows of both chunks) on a
            # SINGLE engine, so the slot release rides one semaphore. When
            # full=False only the first window of each bank is live (odd
            # nseg tail): a 3D AP skips the second halves.
            if full:

                def rng(a):
                    return a  # flat [128, 1024]

                pi = sc[:, 1024 * pb : 1024 * pb + 1024]
                po = ptab[:, 1024 * t : 1024 * t + 1024]
            else:
                pi0 = sc[:, 1024 * pb : 1024 * pb + 256]
                po0 = ptab[:, 1024 * t : 1024 * t + 256]

                def mk(a):
                    return bass.AP(
                        tensor=a.tensor,
                        offset=a.offset,
                        ap=[a.ap[0], [512, 2], [1, 256]],
                    )

                pi = mk(pi0)
                po = mk(po0)
            if dve:
                nc.vector.tensor_scalar(
                    out=po.bitcast(I16),
                    in0=pi,
                    scalar1=C1S,
                    scalar2=C2S,
                    op0=mybir.AluOpType.mult,
                    op1=mybir.AluOpType.add,
                )
            else:
                nc.scalar.activation(
                    po, pi, mybir.ActivationFunctionType.Exp, scale=SCALE
                )

        def emit_trimask(ptab):
            # causal mask on both chunks' diagonal blocks in one op; emitted
            # at flush time (2 chunk-pairs after the exps it reads) so it
            # executes immediately and never head-of-line-blocks the DVE.
            pt0 = ptab[:, 0:128]
            ap2 = bass.AP(
                tensor=pt0.tensor,
                offset=pt0.offset,
                ap=[pt0.ap[0], [512, 2], [1, 128]],
            )
            nc.vector.tensor_mul(
                ap2,
                ap2,
                bass.AP(
                    tensor=t01.tensor,
                    offset=t01.offset,
                    ap=[t01.ap[0], [0, 2], t01.ap[1]],
                ),
            )

        def pv_mms(st, j, ptab):
            acc = st["acc"]
            for i in range(j, NT):
                c = pcol(j, (i - j) * 128)
                # start=True clears the whole PSUM *bank* (4 acc regions), so
                # only the first region touched per bank may set it.
                yield (
                    acc[:, i, 0:PV_N],
                    ptab[:, c : c + 128],
                    st["vp"][:, j, 0:PV_N],
                    j == 0 and i % 4 == 0,
                    j == i,
                )

        def emit_pv_mm(mm):
            o, l, r, st_, sp = mm
            nc.tensor.matmul(o, l, r, start=st_, stop=sp)

        def emit_finish(st, g):
            # normalize/store 8 finished q-tiles (two PSUM banks) straight
            # from PSUM: q-tile i gets its last PV contribution at chunk
            # j=i, so banks 2g,2g+1 are final once chunk 8g+7's PV is done.
            acc = st["acc"]
            g0 = 8 * g
            rec8 = smalls.tile([128, 8], F32, tag="rec")
            nc.vector.reciprocal(rec8, acc[:, g0 : g0 + 8, D])
            osb = outp.tile([128, 8, D], F32, tag="osb")
            r8 = rec8[:, :]
            rec_bcast = bass.AP(
                tensor=r8.tensor,
                offset=r8.offset,
                ap=[r8.ap[0], r8.ap[1], [0, D]],
            )
            nc.vector.tensor_mul(osb, acc[:, g0 : g0 + 8, 0:D], rec_bcast)
            nc.sync.dma_start(out=st["out_r"][:, g0 : g0 + 8, :], in_=osb)

        # ---- one global pipeline over all (pair, chunk-pair) units ----
        states = [None] * PAIRS
        states[0] = load_pair(0)
        pending = []  # (state, ja, jb, ptab) whose PV is not yet emitted
        gseg = 0  # global window counter (slot rotation)
        eunit = 0  # global exp-unit counter (engine assignment)

        def flush_one():
            fst, oa, ob, opab = pending.pop(0)
            emit_trimask(opab)
            pv = list(pv_mms(fst, oa, opab)) + list(pv_mms(fst, ob, opab))
            fin = ob // 8 if ob % 8 == 7 else None
            return fst, pv, fin

        for p in range(PAIRS):
            st = states[p]
            if p + 1 < PAIRS:
                states[p + 1] = load_pair(p + 1)
            st["acc"] = ps_acc.tile([128, NT, 128], F32, tag="acc", name="acc_t")
            for jp in range(0, NT, 2):
                ja, jb = jp, jp + 1
                ptab = ptp.tile([128, 4096], BF16, tag="pt")
                nseg = (S - ja * 128 + 255) // 256
                if gseg % 2 == 1:
                    gseg += 1  # start each chunk-pair on an even window
                s0 = gseg % 4
                pv, fin, fst = [], None, None
                depth = 1 if (p == PAIRS - 1 and jp >= NT - 4) else 2
                if len(pending) >= depth:
                    fst, pv, fin = flush_one()
                nunits = (nseg + 1) // 2
                per_unit = (len(pv) + nunits - 1) // nunits if pv else 0
                k = 0
                for u in range(nseg):
                    # both QK windows of a unit go to the PE queue BEFORE the
                    # PV fill-in batch, so the exp's inputs are never stuck
                    # behind PV matmuls and the exp starts ~0.5us earlier.
                    emit_qk(st, ja, jb, u, ((s0 + u) // 2) % 2)
                    if u % 2 == 1:
                        emit_exp(
                            ptab,
                            u // 2,
                            ((s0 + u) // 2) % 2,
                            True,
                            eunit % 8 in DVE_SEGS,
                        )
                        eunit += 1
                        take = (
                            pv[k : k + per_unit] if u < nseg - 2 else pv[k:]
                        )
                        for mm in take:
                            emit_pv_mm(mm)
                        k += len(take)
                    gseg += 1
                if nseg % 2 == 1:
                    emit_exp(
                        ptab,
                        nseg // 2,
                        ((s0 + nseg - 1) // 2) % 2,
                        False,
                        eunit % 8 in DVE_SEGS,
                    )
                    eunit += 1
                    for mm in pv[k:]:
                        emit_pv_mm(mm)
                    k = len(pv)
                if fin is not None:
                    emit_finish(fst, fin)
                pending.append((st, ja, jb, ptab))
        while pending:
            fst, pv, fin = flush_one()
            for mm in pv:
                emit_pv_mm(mm)
            if fin is not None:
                emit_finish(fst, fin)
    nc.compile()
    return nc


_nc_cache = None


def _get_nc():
    global _nc_cache
    if _nc_cache is None:
        _nc_cache = build_nc()
    return _nc_cache


def kernel(q, k, v, mask):
    """Full causal attention. q,k,v: [B,H,S,D] f32; mask: [1,1,S,S] bool
    (causal tril; baked into the kernel). Returns [B,H,S,D] f32."""
    nc = _get_nc()
    bf = ml_dtypes.bfloat16
    qf = np.asarray(q, dtype=np.float32).reshape(B * H, S, D)
    kf = np.asarray(k, dtype=np.float32).reshape(B * H, S, D)
    vf = np.ascontiguousarray(
        np.asarray(v, dtype=np.float32).reshape(B * H, S, D).astype(bf)
    )
    qTf = np.ascontiguousarray(qf.transpose(0, 2, 1).astype(bf))
    kTf = np.ascontiguousarray(kf.transpose(0, 2, 1).astype(bf))
    in_maps = [
        {
            "qT": qTf[i * PAIRS : (i + 1) * PAIRS],
            "kT": kTf[i * PAIRS : (i + 1) * PAIRS],
            "v": vf[i * PAIRS : (i + 1) * PAIRS],
        }
        for i in range(NCORES)
    ]
    res = run_bass_kernel_spmd(nc, in_maps, core_ids=list(range(NCORES)))
    o = np.concatenate([res.results[i]["out"] for i in range(NCORES)], axis=0)
    return o.reshape(B, H, S, D)
